# revision 10
# baseline (speedup 1.0000x reference)
"""Trainium2 Bass kernel for the BSG word2gauss-style hinge/KL loss.

Strategy (data-parallel over 8 NeuronCores):
  - Host precomputes gather tables (batch-independent weight prep), bf16.
    Key algebra: 2*kl + L = A_b*iv_w + h'_b . g'_w + c_w - lsg_b with
      A_b  = exp(lsg_b) + sum(mu_b^2)
      g'_w = -2*iv_w*(wf[:, :L] @ tm_w)  in R^{H+1}   (projected type mean)
      c_w  = sq_w*iv_w + lv_w
    so the per-(row, word) interaction is a 51-dim dot with h' = [h; 1]
    instead of a 100-dim dot with mu, and no mu2 scaling pass is needed.
    Tables:
      CT [V,128] bf16 (256B rows): 0:50 U = emb@W1[:50], 50:101 g', 101 iv,
         102 c                                         (context stream)
      NT [V, 64] bf16 (128B rows): 0:51 g', 51 iv, 52 c     (neg stream)
      ZT [V,128] bf16: 0:50 Ucen = emb@W1[50:]+b1, rest as CT  (centers)
  - Gathers use dma_gather (SWDGE). Its int16 index limit (<32768 rows) is
    handled by gathering PAIRED rows: index = id>>1 with elem_size = 2 rows,
    then one contiguous parity select keeps the useful low columns.
    <=1024 indices per instruction (SWDGE descriptor-ring capacity), spread
    over 4 queues.
  - Each core processes 8192 batch rows in 16 gather-blocks of 512. Flat
    gather position i -> (partition i%128, slot i//128), so host index
    order is slot-major. Per 128-row sub-block:
      h = sum_j relu(U[ctx_j] + Ucen[cen]);  [h;1] @ [Wmu|Wls;bmu|bls] on PE
      A = exp(logsigma) + sum(mu^2);  dots = h' . g' for ctx/neg/cen
    then kl algebra + hinge on [128,40] vectors, accumulated in f32.
  - Output per core: [128,2] partials; host reduces, applies -L/2, /B.
"""

import sys

for _p in ("/opt/trn_rl_repo", "/opt/pypackages"):
    if _p not in sys.path:
        sys.path.append(_p)

from contextlib import ExitStack

import numpy as np
import ml_dtypes

import concourse.bass as bass
import concourse.tile as tile
from concourse import bacc, mybir
from concourse.bass_utils import run_bass_kernel_spmd
from concourse.masks import make_identity

dt = mybir.dt
F32 = dt.float32
BF16 = dt.bfloat16
AF = mybir.ActivationFunctionType
OP = mybir.AluOpType
AX = mybir.AxisListType

V, D, H, L = 50000, 50, 50, 100
C = 10
B = 65536
NCORES = 8
NB = B // NCORES     # rows per core: 8192
GBS = 512            # rows per gather block
NGB = NB // GBS      # 16
NSB = GBS // 128     # 4 sub-blocks
Q = NSB * C          # 40 ctx slots per partition per gather block
EC = 128             # CT/ZT row width (bf16 elems, 256B)
EN = 64              # NT row width (bf16 elems, 128B)
MAXI = 1024          # max idxs per dma_gather (SWDGE ring capacity)
MARGIN = 1.0
NPAY = 103           # useful bf16 cols in a CT/ZT row (U 50 + g' 51 + iv + c)
NPAYN = 53           # useful bf16 cols in an NT row (g' 51 + iv + c)

_CACHE: dict = {}


def _wrap_idx(flat):
    """int16 idx list -> [128, ceil(n/16)] wrapped-16, replicated across cores."""
    n = len(flat)
    nf = -(-n // 16)
    w = np.zeros((16, nf), np.int16)
    w[np.arange(n) % 16, np.arange(n) // 16] = flat
    return np.tile(w, (8, 1))


def _build_program():
    nc = bacc.Bacc("TRN2", target_bir_lowering=False, debug=False, num_swdge_queues=4)

    ct_d = nc.dram_tensor("ct", [V, EC], BF16, kind="ExternalInput")
    nt_d = nc.dram_tensor("nt", [V, EN], BF16, kind="ExternalInput")
    zt_d = nc.dram_tensor("zt", [V, EC], BF16, kind="ExternalInput")
    wf_d = nc.dram_tensor("wf", [H + 1, L + 1], F32, kind="ExternalInput")
    # wrapped int16 half-indices, concatenated per gather block:
    #   per gb: ctx (Q*128/16 cols) | neg | cen (NSB*128/16 cols)
    IGC = Q * 128 // 16          # 320 idx cols per gb for ctx/neg streams
    IGZ = NSB * 128 // 16        # 32 idx cols per gb for cen stream
    IG = 2 * IGC + IGZ
    idx_d = nc.dram_tensor("idx", [128, NGB * IG], dt.int16, kind="ExternalInput")
    # parity masks (uint8 0/1): per gb: ctx Q | neg Q | cen NSB
    MG = 2 * Q + NSB
    msk_d = nc.dram_tensor("msk", [128, NGB * MG], dt.uint8, kind="ExternalInput")
    out_d = nc.dram_tensor("out", [128, 2], F32, kind="ExternalOutput")

    # paired views: half-row index k -> rows [2k, 2k+1]
    ct_v = bass.AP(ct_d, 0, [[2 * EC, V // 2], [1, 2 * EC]])
    nt_v = bass.AP(nt_d, 0, [[2 * EN, V // 2], [1, 2 * EN]])
    zt_v = bass.AP(zt_d, 0, [[2 * EC, V // 2], [1, 2 * EC]])

    def gather(out_ap, tab_v, idx_ap, n, es):
        nc.gpsimd.dma_gather(
            out_ap=out_ap, in_ap=tab_v, idxs_ap=idx_ap,
            num_idxs=n, num_idxs_reg=n, elem_size=es, elem_step=es,
            queue_num=0)

    with tile.TileContext(nc) as tc, ExitStack() as ctx:
        const = ctx.enter_context(tc.tile_pool(name="const", bufs=1))
        io = ctx.enter_context(tc.tile_pool(name="io", bufs=3))
        wk = ctx.enter_context(tc.tile_pool(name="wk", bufs=2))
        ps = ctx.enter_context(tc.tile_pool(name="ps", bufs=2, space="PSUM"))
        accp = ctx.enter_context(tc.tile_pool(name="accp", bufs=1))

        ident = const.tile([128, 128], F32)
        make_identity(nc, ident[:])
        wf_sb = const.tile([H + 1, L + 1], F32)
        nc.sync.dma_start(wf_sb[:], wf_d.ap())
        idx_sb = const.tile([128, NGB * IG], dt.int16)
        nc.sync.dma_start(idx_sb[:], idx_d.ap())
        msk_sb = const.tile([128, NGB * MG], dt.uint8)
        nc.sync.dma_start(msk_sb[:], msk_d.ap())

        acc_h = accp.tile([128, Q], F32)
        acc_c = accp.tile([128, NSB], F32)
        nc.vector.memset(acc_h[:], 0.0)
        nc.vector.memset(acc_c[:], 0.0)

        for gb in range(NGB):
            PG = io.tile([128, Q, 2 * EC], BF16, tag="PG")    # ctx row pairs
            NG = io.tile([128, Q, 2 * EN], BF16, tag="NG")    # neg row pairs
            CG = io.tile([128, NSB, 2 * EC], BF16, tag="CG")  # cen row pairs

            icx = idx_sb[:, gb * IG:gb * IG + IGC]
            ing = idx_sb[:, gb * IG + IGC:gb * IG + 2 * IGC]
            icn = idx_sb[:, gb * IG + 2 * IGC:(gb + 1) * IG]
            # 1024-idx chunks: chunk g covers slots [g*8, g*8+8).  cen + ctx
            # first (feed the h pipeline); neg drains during the h phase.
            NCH = Q * 128 // MAXI                          # 5
            SCH = MAXI // 128                              # 8 slots per chunk
            gather(CG[:], zt_v, icn, NSB * 128, 2 * EC)
            for g in range(NCH):
                sl = slice(g * SCH, (g + 1) * SCH)
                gather(PG[:, sl, :], ct_v, icx[:, g * 64:(g + 1) * 64], MAXI, 2 * EC)
            for g in range(NCH):
                sl = slice(g * SCH, (g + 1) * SCH)
                gather(NG[:, sl, :], nt_v, ing[:, g * 64:(g + 1) * 64], MAXI, 2 * EN)

            # parity select, in place, on f32-bitcast views (half the lanes):
            # keep the chosen row in cols [0:NPAY)
            mc = msk_sb[:, gb * MG:gb * MG + Q]
            mn = msk_sb[:, gb * MG + Q:gb * MG + 2 * Q]
            mz = msk_sb[:, gb * MG + 2 * Q:(gb + 1) * MG]
            NP2, NPN2 = (NPAY + 1) // 2, (NPAYN + 1) // 2
            nc.vector.copy_predicated(CG[:, :, 0:2 * NP2].bitcast(F32),
                                      mz.unsqueeze(2).to_broadcast([128, NSB, NP2]),
                                      CG[:, :, EC:EC + 2 * NP2].bitcast(F32))
            nc.vector.copy_predicated(PG[:, :, 0:2 * NP2].bitcast(F32),
                                      mc.unsqueeze(2).to_broadcast([128, Q, NP2]),
                                      PG[:, :, EC:EC + 2 * NP2].bitcast(F32))

            PG4 = PG[:].rearrange("p (s c) e -> p s c e", s=NSB)
            NG4 = NG[:].rearrange("p (s c) e -> p s c e", s=NSB)

            # h = sum_j relu(U_ctx + U_cen), one batched pass per block
            y4 = wk.tile([128, NSB, C, D], BF16, tag="y4")
            nc.vector.tensor_tensor(
                out=y4[:], in0=PG4[:, :, :, 0:D],
                in1=CG[:, :, 0:D].unsqueeze(2).to_broadcast([128, NSB, C, D]),
                op=OP.add)
            r4 = wk.tile([128, NSB, C, D], BF16, tag="r4")
            nc.vector.tensor_scalar_max(out=r4[:], in0=y4[:], scalar1=0.0)
            h4 = wk.tile([128, NSB, H + 1], F32, tag="h4")
            nc.vector.tensor_reduce(out=h4[:, :, 0:D],
                                    in_=r4[:].transpose([0, 1, 3, 2]),
                                    axis=AX.X, op=OP.add)
            nc.vector.memset(h4[:, :, H:H + 1], 1.0)
            hb4 = wk.tile([128, NSB, H + 1], BF16, tag="hb4")
            nc.scalar.copy(hb4[:], h4[:])

            # mu = h' @ wf on PE, per sub-block; batched epilogue
            hT_ps = ps.tile([64, NSB * 128], F32, tag="hTp")
            for s in range(NSB):
                nc.tensor.transpose(hT_ps[0:H + 1, s * 128:(s + 1) * 128],
                                    h4[:, s, :], ident[:])
            hT = wk.tile([64, NSB * 128], F32, tag="hT")
            nc.scalar.copy(hT[0:H + 1, :], hT_ps[0:H + 1, :])
            mu_ps = ps.tile([128, NSB, L + 1], F32, tag="mu")
            for s in range(NSB):
                nc.tensor.matmul(mu_ps[:, s, :],
                                 lhsT=hT[0:H + 1, s * 128:(s + 1) * 128],
                                 rhs=wf_sb[:], start=True, stop=True)
            A_t = wk.tile([128, NSB], F32, tag="A")
            sqj = wk.tile([128, L], F32, tag="sqj")
            for s in range(NSB):
                nc.scalar.activation(sqj[:], mu_ps[:, s, 0:L], AF.Square,
                                     accum_out=A_t[:, s:s + 1])
            sig = wk.tile([128, NSB], F32, tag="sig")
            nc.scalar.activation(sig[:], mu_ps[:, :, L], AF.Exp)
            lsg_t = wk.tile([128, NSB], F32, tag="lsg")
            nc.scalar.copy(lsg_t[:], mu_ps[:, :, L])

            # neg select (no dep on the scalar chain), then cen dot, then the
            # fused ctx-neg difference dot:
            # dc - dn = ((gc - gn) . h')  -- one mult+reduce instead of two
            nc.vector.copy_predicated(NG[:, :, 0:2 * NPN2].bitcast(F32),
                                      mn.unsqueeze(2).to_broadcast([128, Q, NPN2]),
                                      NG[:, :, EN:EN + 2 * NPN2].bitcast(F32))
            hbb = hb4[:].unsqueeze(2).to_broadcast([128, NSB, C, H + 1])
            cd = wk.tile([128, NSB], F32, tag="cd")
            pz = wk.tile([128, NSB, H + 1], BF16, tag="pz")
            nc.vector.tensor_tensor(out=pz[:], in0=CG[:, :, D:D + H + 1],
                                    in1=hb4[:], op=OP.mult)
            nc.vector.tensor_reduce(out=cd[:], in_=pz[:], axis=AX.X, op=OP.add)
            gd = wk.tile([128, NSB, C, H + 1], BF16, tag="gd")
            nc.vector.tensor_tensor(out=gd[:], in0=PG4[:, :, :, D:D + H + 1],
                                    in1=NG4[:, :, :, 0:H + 1], op=OP.subtract)
            pd = wk.tile([128, NSB, C, H + 1], BF16, tag="pd")
            nc.vector.tensor_tensor(out=pd[:], in0=gd[:], in1=hbb, op=OP.mult)
            v1 = wk.tile([128, NSB, C], F32, tag="v1")
            nc.vector.tensor_reduce(out=v1[:], in_=pd[:], axis=AX.X, op=OP.add)

            # hinge: d = (dc-dn) + (cc-cn) + A*(ivc-ivn); relu(0.5*d + 1)
            v2 = wk.tile([128, NSB, C], F32, tag="v2")
            nc.vector.tensor_tensor(out=v2[:], in0=PG4[:, :, :, D + H + 2],
                                    in1=NG4[:, :, :, H + 2], op=OP.subtract)
            v3 = wk.tile([128, NSB, C], F32, tag="v3")
            nc.vector.tensor_tensor(out=v3[:], in0=PG4[:, :, :, D + H + 1],
                                    in1=NG4[:, :, :, H + 1], op=OP.subtract)
            nc.vector.tensor_tensor(out=A_t[:], in0=A_t[:], in1=sig[:], op=OP.add)
            nc.vector.tensor_tensor(
                out=v3[:], in0=v3[:],
                in1=A_t[:].unsqueeze(2).to_broadcast([128, NSB, C]), op=OP.mult)
            nc.vector.tensor_tensor(out=v1[:], in0=v1[:], in1=v2[:], op=OP.add)
            nc.vector.tensor_tensor(out=v1[:], in0=v1[:], in1=v3[:], op=OP.add)
            hng = wk.tile([128, Q], F32, tag="hng")
            nc.scalar.activation(hng[:].rearrange("p (s c) -> p s c", s=NSB), v1[:],
                                 AF.Relu, bias=float(MARGIN), scale=0.5)
            nc.vector.tensor_tensor(out=acc_h[:], in0=acc_h[:], in1=hng[:], op=OP.add)

            cw = wk.tile([128, NSB], F32, tag="cw")
            nc.vector.tensor_tensor(out=cw[:], in0=cd[:], in1=CG[:, :, D + H + 2],
                                    op=OP.add)
            ca = wk.tile([128, NSB], F32, tag="ca")
            nc.vector.tensor_tensor(out=ca[:], in0=CG[:, :, D + H + 1], in1=A_t[:],
                                    op=OP.mult)
            nc.vector.tensor_tensor(out=cw[:], in0=cw[:], in1=ca[:], op=OP.add)
            nc.vector.tensor_tensor(out=cw[:], in0=cw[:], in1=lsg_t[:], op=OP.subtract)
            nc.vector.tensor_tensor(out=acc_c[:], in0=acc_c[:], in1=cw[:], op=OP.add)

        outt = accp.tile([128, 2], F32)
        nc.vector.tensor_reduce(out=outt[:, 0:1], in_=acc_h[:], axis=AX.X, op=OP.add)
        nc.vector.tensor_reduce(out=outt[:, 1:2], in_=acc_c[:], axis=AX.X, op=OP.add)
        nc.sync.dma_start(out_d.ap(), outt[:])

    # Spread gathers across the 4 SWDGE queues (4 Q7 core-pairs run desc-gen
    # in parallel). queue = Tile-assigned DMASW sem lane % 4 keeps per-lane
    # completion FIFO within its queue, so Tile's sem ordering stays sound.
    import re
    for inst in nc.inst_map.values():
        if type(inst).__name__ == "InstDMAGatherAnt" and inst.sync_info:
            for u in inst.sync_info.on_update:
                m = re.match(r"DMASW(\d+)_", u.ant_name or "")
                if m:
                    inst.queue_num = int(m.group(1)) % 4
                    break
    nc.compile()
    return nc


def _prep_inputs(emb, W1, b1, Wmu, bmu, Wls, bls, type_means_tbl,
                 type_logvars_tbl, centers, contexts, neg_contexts):
    emb = np.asarray(emb, np.float32)
    W1 = np.asarray(W1, np.float32)
    U = emb @ W1[:D]
    Ucen = emb @ W1[D:] + np.asarray(b1, np.float32)

    tm = np.asarray(type_means_tbl, np.float32)
    lv = np.asarray(type_logvars_tbl, np.float32)[:, 0]
    sq = (tm * tm).sum(axis=1)
    iv = np.exp(-lv)

    wf = np.zeros((H + 1, L + 1), np.float32)
    wf[0:H, 0:L] = np.asarray(Wmu, np.float32)
    wf[0:H, L] = np.asarray(Wls, np.float32)[:, 0]
    wf[H, 0:L] = np.asarray(bmu, np.float32)
    wf[H, L] = np.asarray(bls, np.float32)[0]

    G = (tm @ wf[0:H + 1, 0:L].T) * (-2.0 * iv)[:, None]    # [V, H+1]
    c = sq * iv + lv

    ct = np.zeros((V, EC), np.float32)
    ct[:, 0:D] = U
    ct[:, D:D + H + 1] = G
    ct[:, D + H + 1] = iv
    ct[:, D + H + 2] = c
    zt = ct.copy()
    zt[:, 0:D] = Ucen
    nt = np.zeros((V, EN), np.float32)
    nt[:, 0:H + 1] = G
    nt[:, H + 1] = iv
    nt[:, H + 2] = c
    ct = ct.astype(ml_dtypes.bfloat16)
    zt = zt.astype(ml_dtypes.bfloat16)
    nt = nt.astype(ml_dtypes.bfloat16)

    # flat gather order: position i = slot*128 + p; slot = s*C + j for ctx/neg,
    # slot = s for cen; b = core*NB + gb*GBS + s*128 + p
    cx = np.asarray(contexts, np.int32).reshape(NCORES, NGB, NSB, 128, C)
    ng = np.asarray(neg_contexts, np.int32).reshape(NCORES, NGB, NSB, 128, C)
    cn = np.asarray(centers, np.int32).reshape(NCORES, NGB, NSB, 128)
    # -> [core, gb, slot(s,j), p] flat per stream
    cxf = cx.transpose(0, 1, 2, 4, 3).reshape(NCORES, NGB, Q * 128)
    ngf = ng.transpose(0, 1, 2, 4, 3).reshape(NCORES, NGB, Q * 128)
    cnf = cn.reshape(NCORES, NGB, NSB * 128)

    in_maps = []
    for cix in range(NCORES):
        iparts, mparts = [], []
        for gb in range(NGB):
            for f in (cxf[cix, gb], ngf[cix, gb], cnf[cix, gb]):
                iparts.append(_wrap_idx((f >> 1).astype(np.int16)))
            # masks in [p, slot] layout
            mparts.append(np.ascontiguousarray(
                (cxf[cix, gb] & 1).reshape(Q, 128).T.astype(np.uint8)))
            mparts.append(np.ascontiguousarray(
                (ngf[cix, gb] & 1).reshape(Q, 128).T.astype(np.uint8)))
            mparts.append(np.ascontiguousarray(
                (cnf[cix, gb] & 1).reshape(NSB, 128).T.astype(np.uint8)))
        in_maps.append({
            "ct": ct, "nt": nt, "zt": zt, "wf": wf,
            "idx": np.concatenate(iparts, axis=1),
            "msk": np.concatenate(mparts, axis=1),
        })
    return in_maps


def kernel(**inputs) -> np.ndarray:
    if "nc" not in _CACHE:
        _CACHE["nc"] = _build_program()
    nc = _CACHE["nc"]
    in_maps = _prep_inputs(**inputs)
    res = run_bass_kernel_spmd(nc, in_maps, core_ids=list(range(NCORES)))
    total = 0.0
    for cix in range(NCORES):
        out = np.asarray(res.results[cix]["out"], np.float64)
        total += out[:, 0].sum() + 0.5 * out[:, 1].sum()
    loss = total / B - L / 2.0
    return np.float32(loss)


# revision 12
# speedup vs baseline: 1.0506x; 1.0506x over previous
"""Trainium2 Bass kernel for the BSG word2gauss-style hinge/KL loss.

Strategy (data-parallel over 8 NeuronCores):
  - Host precomputes gather tables (batch-independent weight prep).
    Key algebra: 2*kl + L = A_b*iv_w + h'_b . g'_w + c_w - lsg_b with
      A_b  = exp(lsg_b) + sum(mu_b^2)
      g'_w = -2*iv_w*(wf[:, :L] @ tm_w)  in R^{H+1}   (projected type mean)
      c_w  = sq_w*iv_w + lv_w
    so the per-(row, word) interaction is a 51-dim dot with h' = [h; 1]
    instead of a 100-dim dot with mu, and no mu2 scaling pass is needed.
    Tables, 128B rows (fp8 e4m3 payload + bf16 scalars), byte layout:
      CT/ZT [V, 64 bf16]: 0:50 U/Ucen fp8, 50:101 g' fp8, 102:104 iv bf16,
                          104:106 c bf16
      NT    [V, 64 bf16]: 0:51 g' fp8, 52:54 iv bf16, 54:56 c bf16
    (fp8 on U/g' costs ~3e-4 relative loss error, far under the 2e-2 gate,
    and halves gather payload: 256B paired elements at ~17.4ns/descriptor.)
  - Gathers use dma_gather (SWDGE). Its int16 index limit (<32768 rows) is
    handled by gathering PAIRED rows: index = id>>1 with elem_size = 2 rows,
    then one contiguous parity select (on f32-bitcast lanes, half the
    elements) keeps the useful low bytes. <=1024 indices per instruction
    (SWDGE descriptor-ring capacity), spread over 4 queues.
  - Each core processes 8192 batch rows in 16 gather-blocks of 512. Flat
    gather position i -> (partition i%128, slot i//128), so host index
    order is slot-major. Per gather block, batched over 4 sub-blocks:
      h = sum_j relu(U[ctx_j] + Ucen[cen]);  [h;1] @ [Wmu|Wls;bmu|bls] on PE
      A = exp(logsigma) + sum(mu^2)
      dc - dn = (g'_ctx - g'_neg) . h'   (fused difference dot)
    then hinge/cen algebra on [128,40] vectors, accumulated in f32.
  - Output per core: [128,2] partials; host reduces, applies -L/2, /B.
"""

import sys

for _p in ("/opt/trn_rl_repo", "/opt/pypackages"):
    if _p not in sys.path:
        sys.path.append(_p)

from contextlib import ExitStack

import numpy as np
import ml_dtypes

import concourse.bass as bass
import concourse.tile as tile
from concourse import bacc, mybir
from concourse.bass_utils import run_bass_kernel_spmd
from concourse.masks import make_identity

dt = mybir.dt
F32 = dt.float32
BF16 = dt.bfloat16
F8 = dt.float8e4
AF = mybir.ActivationFunctionType
OP = mybir.AluOpType
AX = mybir.AxisListType

V, D, H, L = 50000, 50, 50, 100
C = 10
B = 65536
NCORES = 8
NB = B // NCORES     # rows per core: 8192
GBS = 512            # rows per gather block
NGB = NB // GBS      # 16
NSB = GBS // 128     # 4 sub-blocks
Q = NSB * C          # 40 ctx slots per partition per gather block
E = 64               # table row width (bf16 elems, 128B)
MAXI = 1024          # max idxs per dma_gather (SWDGE ring capacity)
MARGIN = 1.0
# f32-lane select widths (payload bytes / 4, rounded up)
SELW = 27            # CT/ZT payload 106B
SELWN = 14           # NT payload 56B
IVC, CC = 51, 52     # bf16 col of iv/c in CT/ZT rows
IVN, CN = 26, 27     # bf16 col of iv/c in NT rows

_CACHE: dict = {}


def _wrap_idx(flat):
    """int16 idx list -> [128, ceil(n/16)] wrapped-16, replicated across cores."""
    n = len(flat)
    nf = -(-n // 16)
    w = np.zeros((16, nf), np.int16)
    w[np.arange(n) % 16, np.arange(n) // 16] = flat
    return np.tile(w, (8, 1))


def _build_program():
    nc = bacc.Bacc("TRN2", target_bir_lowering=False, debug=False, num_swdge_queues=4)

    ct_d = nc.dram_tensor("ct", [V, E], BF16, kind="ExternalInput")
    nt_d = nc.dram_tensor("nt", [V, E], BF16, kind="ExternalInput")
    zt_d = nc.dram_tensor("zt", [V, E], BF16, kind="ExternalInput")
    wf_d = nc.dram_tensor("wf", [H + 1, L + 1], F32, kind="ExternalInput")
    # wrapped int16 half-indices, concatenated per gather block:
    #   per gb: ctx (Q*128/16 cols) | neg | cen (NSB*128/16 cols)
    IGC = Q * 128 // 16          # 320 idx cols per gb for ctx/neg streams
    IGZ = NSB * 128 // 16        # 32 idx cols per gb for cen stream
    IG = 2 * IGC + IGZ
    idx_d = nc.dram_tensor("idx", [128, NGB * IG], dt.int16, kind="ExternalInput")
    # parity masks (uint8 0/1): per gb: ctx Q | neg Q | cen NSB
    MG = 2 * Q + NSB
    msk_d = nc.dram_tensor("msk", [128, NGB * MG], dt.uint8, kind="ExternalInput")
    out_d = nc.dram_tensor("out", [128, 2], F32, kind="ExternalOutput")

    # paired views: half-row index k -> rows [2k, 2k+1] (256B elements)
    ct_v = bass.AP(ct_d, 0, [[2 * E, V // 2], [1, 2 * E]])
    nt_v = bass.AP(nt_d, 0, [[2 * E, V // 2], [1, 2 * E]])
    zt_v = bass.AP(zt_d, 0, [[2 * E, V // 2], [1, 2 * E]])

    def gather(out_ap, tab_v, idx_ap, n):
        nc.gpsimd.dma_gather(
            out_ap=out_ap, in_ap=tab_v, idxs_ap=idx_ap,
            num_idxs=n, num_idxs_reg=n, elem_size=2 * E, elem_step=2 * E,
            queue_num=0)

    with tile.TileContext(nc) as tc, ExitStack() as ctx:
        const = ctx.enter_context(tc.tile_pool(name="const", bufs=1))
        io = ctx.enter_context(tc.tile_pool(name="io", bufs=4))
        wk = ctx.enter_context(tc.tile_pool(name="wk", bufs=2))
        ps = ctx.enter_context(tc.tile_pool(name="ps", bufs=2, space="PSUM"))
        accp = ctx.enter_context(tc.tile_pool(name="accp", bufs=1))

        ident = const.tile([128, 128], F32)
        make_identity(nc, ident[:])
        wf_sb = const.tile([H + 1, L + 1], F32)
        nc.sync.dma_start(wf_sb[:], wf_d.ap())
        idx_sb = const.tile([128, NGB * IG], dt.int16)
        nc.sync.dma_start(idx_sb[:], idx_d.ap())
        msk_sb = const.tile([128, NGB * MG], dt.uint8)
        nc.sync.dma_start(msk_sb[:], msk_d.ap())

        acc_h = accp.tile([128, Q], F32)
        acc_c = accp.tile([128, NSB], F32)
        nc.vector.memset(acc_h[:], 0.0)
        nc.vector.memset(acc_c[:], 0.0)

        for gb in range(NGB):
            PG = io.tile([128, Q, 2 * E], BF16, tag="PG")     # ctx row pairs
            NG = io.tile([128, Q, 2 * E], BF16, tag="NG")     # neg row pairs
            CG = io.tile([128, NSB, 2 * E], BF16, tag="CG")   # cen row pairs

            icx = idx_sb[:, gb * IG:gb * IG + IGC]
            ing = idx_sb[:, gb * IG + IGC:gb * IG + 2 * IGC]
            icn = idx_sb[:, gb * IG + 2 * IGC:(gb + 1) * IG]
            # 1024-idx chunks: chunk g covers slots [g*8, g*8+8).  cen + ctx
            # first (feed the h pipeline); neg drains during the h phase.
            NCH = Q * 128 // MAXI                          # 5
            SCH = MAXI // 128                              # 8 slots per chunk
            gather(CG[:], zt_v, icn, NSB * 128)
            for g in range(NCH):
                sl = slice(g * SCH, (g + 1) * SCH)
                gather(PG[:, sl, :], ct_v, icx[:, g * 64:(g + 1) * 64], MAXI)
            for g in range(NCH):
                sl = slice(g * SCH, (g + 1) * SCH)
                gather(NG[:, sl, :], nt_v, ing[:, g * 64:(g + 1) * 64], MAXI)

            # parity select, in place, on f32-bitcast lanes:
            # keep the chosen row's payload in the low half
            mc = msk_sb[:, gb * MG:gb * MG + Q]
            mn = msk_sb[:, gb * MG + Q:gb * MG + 2 * Q]
            mz = msk_sb[:, gb * MG + 2 * Q:(gb + 1) * MG]
            nc.vector.copy_predicated(
                CG[:, :, 0:2 * SELW].bitcast(F32),
                mz.unsqueeze(2).to_broadcast([128, NSB, SELW]),
                CG[:, :, E:E + 2 * SELW].bitcast(F32))
            nc.vector.copy_predicated(
                PG[:, :, 0:2 * SELW].bitcast(F32),
                mc.unsqueeze(2).to_broadcast([128, Q, SELW]),
                PG[:, :, E:E + 2 * SELW].bitcast(F32))

            PG4 = PG[:].rearrange("p (s c) e -> p s c e", s=NSB)
            NG4 = NG[:].rearrange("p (s c) e -> p s c e", s=NSB)
            PG84 = PG[:].bitcast(F8).rearrange("p (s c) e -> p s c e", s=NSB)
            NG84 = NG[:].bitcast(F8).rearrange("p (s c) e -> p s c e", s=NSB)
            CG8 = CG[:].bitcast(F8)

            # h = sum_j relu(U_ctx + U_cen), one batched pass per block
            y4 = wk.tile([128, NSB, C, D], BF16, tag="y4")
            nc.vector.tensor_tensor(
                out=y4[:], in0=PG84[:, :, :, 0:D],
                in1=CG8[:, :, 0:D].unsqueeze(2).to_broadcast([128, NSB, C, D]),
                op=OP.add)
            r4 = wk.tile([128, NSB, C, D], BF16, tag="r4")
            nc.scalar.activation(r4[:], y4[:], AF.Relu)
            h4 = wk.tile([128, NSB, H + 1], F32, tag="h4")
            nc.vector.tensor_reduce(out=h4[:, :, 0:D],
                                    in_=r4[:].transpose([0, 1, 3, 2]),
                                    axis=AX.X, op=OP.add)
            nc.vector.memset(h4[:, :, H:H + 1], 1.0)
            hb4 = wk.tile([128, NSB, H + 1], BF16, tag="hb4")
            nc.scalar.copy(hb4[:], h4[:])

            # mu = h' @ wf on PE, per sub-block; batched epilogue
            hT_ps = ps.tile([64, NSB * 128], F32, tag="hTp")
            for s in range(NSB):
                nc.tensor.transpose(hT_ps[0:H + 1, s * 128:(s + 1) * 128],
                                    h4[:, s, :], ident[:])
            hT = wk.tile([64, NSB * 128], F32, tag="hT")
            nc.scalar.copy(hT[0:H + 1, :], hT_ps[0:H + 1, :])
            mu_ps = ps.tile([128, NSB, L + 1], F32, tag="mu")
            for s in range(NSB):
                nc.tensor.matmul(mu_ps[:, s, :],
                                 lhsT=hT[0:H + 1, s * 128:(s + 1) * 128],
                                 rhs=wf_sb[:], start=True, stop=True)
            A_t = wk.tile([128, NSB], F32, tag="A")
            sqj = wk.tile([128, L], F32, tag="sqj")
            for s in range(NSB):
                nc.scalar.activation(sqj[:], mu_ps[:, s, 0:L], AF.Square,
                                     accum_out=A_t[:, s:s + 1])
            sig = wk.tile([128, NSB], F32, tag="sig")
            nc.scalar.activation(sig[:], mu_ps[:, :, L], AF.Exp)
            lsg_t = wk.tile([128, NSB], F32, tag="lsg")
            nc.scalar.copy(lsg_t[:], mu_ps[:, :, L])
            gz = wk.tile([128, NSB, H + 1], BF16, tag="gz")
            nc.scalar.copy(gz[:], CG8[:, :, D:D + H + 1])

            # neg select (no dep on the scalar chain), then cen dot, then the
            # fused ctx-neg difference dot:
            # dc - dn = ((gc - gn) . h')  -- one mult+reduce instead of two
            nc.vector.copy_predicated(
                NG[:, :, 0:2 * SELWN].bitcast(F32),
                mn.unsqueeze(2).to_broadcast([128, Q, SELWN]),
                NG[:, :, E:E + 2 * SELWN].bitcast(F32))
            hbb = hb4[:].unsqueeze(2).to_broadcast([128, NSB, C, H + 1])
            cd = wk.tile([128, NSB], F32, tag="cd")
            pz = wk.tile([128, NSB, H + 1], BF16, tag="pz")
            nc.vector.tensor_tensor(out=pz[:], in0=gz[:], in1=hb4[:], op=OP.mult)
            nc.vector.tensor_reduce(out=cd[:], in_=pz[:], axis=AX.X, op=OP.add)
            gd = wk.tile([128, NSB, C, H + 1], BF16, tag="gd")
            nc.vector.tensor_tensor(out=gd[:], in0=PG84[:, :, :, D:D + H + 1],
                                    in1=NG84[:, :, :, 0:H + 1], op=OP.subtract)
            pd = wk.tile([128, NSB, C, H + 1], BF16, tag="pd")
            nc.vector.tensor_tensor(out=pd[:], in0=gd[:], in1=hbb, op=OP.mult)
            v1 = wk.tile([128, NSB, C], F32, tag="v1")
            nc.vector.tensor_reduce(out=v1[:], in_=pd[:], axis=AX.X, op=OP.add)

            # hinge: d = (dc-dn) + (cc-cn) + A*(ivc-ivn); relu(0.5*d + 1)
            v2 = wk.tile([128, NSB, C], F32, tag="v2")
            nc.vector.tensor_tensor(out=v2[:], in0=PG4[:, :, :, CC],
                                    in1=NG4[:, :, :, CN], op=OP.subtract)
            v3 = wk.tile([128, NSB, C], F32, tag="v3")
            nc.vector.tensor_tensor(out=v3[:], in0=PG4[:, :, :, IVC],
                                    in1=NG4[:, :, :, IVN], op=OP.subtract)
            nc.vector.tensor_tensor(out=A_t[:], in0=A_t[:], in1=sig[:], op=OP.add)
            nc.vector.tensor_tensor(
                out=v3[:], in0=v3[:],
                in1=A_t[:].unsqueeze(2).to_broadcast([128, NSB, C]), op=OP.mult)
            nc.vector.tensor_tensor(out=v1[:], in0=v1[:], in1=v2[:], op=OP.add)
            nc.vector.tensor_tensor(out=v1[:], in0=v1[:], in1=v3[:], op=OP.add)
            hng = wk.tile([128, Q], F32, tag="hng")
            nc.scalar.activation(hng[:].rearrange("p (s c) -> p s c", s=NSB), v1[:],
                                 AF.Relu, bias=float(MARGIN), scale=0.5)
            nc.vector.tensor_tensor(out=acc_h[:], in0=acc_h[:], in1=hng[:], op=OP.add)

            cw = wk.tile([128, NSB], F32, tag="cw")
            nc.vector.tensor_tensor(out=cw[:], in0=cd[:], in1=CG[:, :, CC],
                                    op=OP.add)
            ca = wk.tile([128, NSB], F32, tag="ca")
            nc.vector.tensor_tensor(out=ca[:], in0=CG[:, :, IVC], in1=A_t[:],
                                    op=OP.mult)
            nc.vector.tensor_tensor(out=cw[:], in0=cw[:], in1=ca[:], op=OP.add)
            nc.vector.tensor_tensor(out=cw[:], in0=cw[:], in1=lsg_t[:], op=OP.subtract)
            nc.vector.tensor_tensor(out=acc_c[:], in0=acc_c[:], in1=cw[:], op=OP.add)

        outt = accp.tile([128, 2], F32)
        nc.vector.tensor_reduce(out=outt[:, 0:1], in_=acc_h[:], axis=AX.X, op=OP.add)
        nc.vector.tensor_reduce(out=outt[:, 1:2], in_=acc_c[:], axis=AX.X, op=OP.add)
        nc.sync.dma_start(out_d.ap(), outt[:])

    # Spread gathers across the 4 SWDGE queues (4 Q7 core-pairs run desc-gen
    # in parallel). queue = Tile-assigned DMASW sem lane % 4 keeps per-lane
    # completion FIFO within its queue, so Tile's sem ordering stays sound.
    import re
    for inst in nc.inst_map.values():
        if type(inst).__name__ == "InstDMAGatherAnt" and inst.sync_info:
            for u in inst.sync_info.on_update:
                m = re.match(r"DMASW(\d+)_", u.ant_name or "")
                if m:
                    inst.queue_num = int(m.group(1)) % 4
                    break
    nc.compile()
    return nc


def _prep_inputs(emb, W1, b1, Wmu, bmu, Wls, bls, type_means_tbl,
                 type_logvars_tbl, centers, contexts, neg_contexts):
    emb = np.asarray(emb, np.float32)
    W1 = np.asarray(W1, np.float32)
    U = emb @ W1[:D]
    Ucen = emb @ W1[D:] + np.asarray(b1, np.float32)

    tm = np.asarray(type_means_tbl, np.float32)
    lv = np.asarray(type_logvars_tbl, np.float32)[:, 0]
    sq = (tm * tm).sum(axis=1)
    iv = np.exp(-lv)

    wf = np.zeros((H + 1, L + 1), np.float32)
    wf[0:H, 0:L] = np.asarray(Wmu, np.float32)
    wf[0:H, L] = np.asarray(Wls, np.float32)[:, 0]
    wf[H, 0:L] = np.asarray(bmu, np.float32)
    wf[H, L] = np.asarray(bls, np.float32)[0]

    G = (tm @ wf[0:H + 1, 0:L].T) * (-2.0 * iv)[:, None]    # [V, H+1]
    c = sq * iv + lv

    u8 = lambda x: x.astype(ml_dtypes.float8_e4m3).view(np.uint8)
    b8 = lambda x: x.astype(ml_dtypes.bfloat16).view(np.uint8)
    ctb = np.zeros((V, 2 * E), np.uint8)
    ctb[:, 0:D] = u8(U)
    ctb[:, D:D + H + 1] = u8(G)
    ctb[:, 2 * IVC:2 * IVC + 2] = b8(iv).reshape(V, 2)
    ctb[:, 2 * CC:2 * CC + 2] = b8(c).reshape(V, 2)
    ztb = ctb.copy()
    ztb[:, 0:D] = u8(Ucen)
    ntb = np.zeros((V, 2 * E), np.uint8)
    ntb[:, 0:H + 1] = u8(G)
    ntb[:, 2 * IVN:2 * IVN + 2] = b8(iv).reshape(V, 2)
    ntb[:, 2 * CN:2 * CN + 2] = b8(c).reshape(V, 2)
    ct = ctb.view(ml_dtypes.bfloat16)
    zt = ztb.view(ml_dtypes.bfloat16)
    nt = ntb.view(ml_dtypes.bfloat16)

    # flat gather order: position i = slot*128 + p; slot = s*C + j for ctx/neg,
    # slot = s for cen; b = core*NB + gb*GBS + s*128 + p
    cx = np.asarray(contexts, np.int32).reshape(NCORES, NGB, NSB, 128, C)
    ng = np.asarray(neg_contexts, np.int32).reshape(NCORES, NGB, NSB, 128, C)
    cn = np.asarray(centers, np.int32).reshape(NCORES, NGB, NSB, 128)
    # -> [core, gb, slot(s,j), p] flat per stream
    cxf = cx.transpose(0, 1, 2, 4, 3).reshape(NCORES, NGB, Q * 128)
    ngf = ng.transpose(0, 1, 2, 4, 3).reshape(NCORES, NGB, Q * 128)
    cnf = cn.reshape(NCORES, NGB, NSB * 128)

    in_maps = []
    for cix in range(NCORES):
        iparts, mparts = [], []
        for gb in range(NGB):
            for f in (cxf[cix, gb], ngf[cix, gb], cnf[cix, gb]):
                iparts.append(_wrap_idx((f >> 1).astype(np.int16)))
            # masks in [p, slot] layout
            mparts.append(np.ascontiguousarray(
                (cxf[cix, gb] & 1).reshape(Q, 128).T.astype(np.uint8)))
            mparts.append(np.ascontiguousarray(
                (ngf[cix, gb] & 1).reshape(Q, 128).T.astype(np.uint8)))
            mparts.append(np.ascontiguousarray(
                (cnf[cix, gb] & 1).reshape(NSB, 128).T.astype(np.uint8)))
        in_maps.append({
            "ct": ct, "nt": nt, "zt": zt, "wf": wf,
            "idx": np.concatenate(iparts, axis=1),
            "msk": np.concatenate(mparts, axis=1),
        })
    return in_maps


def kernel(**inputs) -> np.ndarray:
    if "nc" not in _CACHE:
        _CACHE["nc"] = _build_program()
    nc = _CACHE["nc"]
    in_maps = _prep_inputs(**inputs)
    res = run_bass_kernel_spmd(nc, in_maps, core_ids=list(range(NCORES)))
    total = 0.0
    for cix in range(NCORES):
        out = np.asarray(res.results[cix]["out"], np.float64)
        total += out[:, 0].sum() + 0.5 * out[:, 1].sum()
    loss = total / B - L / 2.0
    return np.float32(loss)
